# revision 19
# baseline (speedup 1.0000x reference)
"""Additive (Bahdanau) attention on 8 TRN2 NeuronCores.

Reference (B=4, Q=256, K=1024, D=512, H=128):
    qh = q @ w_q.T; kh = k @ w_k.T
    scores[b,q,k] = sum_h w_v[h] * tanh(qh[b,q,h] + kh[b,k,h])
    scores masked to -1e6 for k >= valid_lens[b]; softmax over k; out = attn @ v.

Fourier factorization: tanh(s) ~= sum_{m<=M} c_m sin(m*w*s) (weighted lstsq
fit on the data range); the angle-sum identity makes the [Q,K,H] elementwise
tensor separable into per-harmonic sin/cos features contracted over H on the
PE. M=8 with (PSCALE=1.0, WFLOOR=3e-4): rel err 1.127e-2 measured ==
emulated (gate 2e-2; M=9 lands 9.2e-3 at ~+0.7us).

Key structure (v1 32.6us -> 30.3us / 29.3us mean):
  * K-sliced sharding: valid key ranges cut into <=8 width-288 slices; each
    core computes full-Q bf16 partials [256, 513] = [num | den]; host sums
    slices per batch in fp64 and divides. Softmax shift is the constant
    c_shift so partial exp sums combine exactly.
  * fp8 DoubleRow matmuls for harmonics m>=2: Q/K feature (sin,cos) pairs
    packed along the middle dim of [H, 2, *] APs; one MM per (pair, qblock)
    does a 256-deep contraction in 288 column-cycles (2x bf16). m=1 stays
    bf16 (largest c_m dominates the noise budget). qf pre-scaled by 64 to
    clear the e4m3 denormal floor; exp un-scales via its scale immediate.
  * K-side features built on device: VE fixed-point range reduction
    (u = x*c+MAGIC rounds in the fp32 mantissa; AND keeps exponent + low
    FB bits; ACT Sin's scale/bias maps back to [-pi,pi]). Host-side fp16
    phase shipping was tried and is SLOWER: 1.3MB of phases through the 3
    trigger queues (~190GB/s each) DMA-starves the Sin chain.
  * Activation bias consts are raw (untracked) SBUF tensors memset on
    gpsimd: a Tile-tracked bias adds a second wait-sem per Sin, which the
    multiwait fix turns into queue-stalling EventSemaphores (blocks the
    head Sin's act-table load from overlapping the kh DMA wait). The
    memsets retire >1.4us before the first possible ACT bias read.
  * DMA triggers on sync/gpsimd only (scalar carries none early): ACT's
    first queue entry is the first Sin, so the Sin act-table load runs at
    ~6.7us, inside the kh DMA latency.
  * PE p-state: the clock ramps to 2.4GHz only after a long CONTINUOUS
    busy run (~11us from first PE op); idle gaps reset it. Warm matmuls
    on a memset tile start the PE at ~7.2us; filler warms plug the
    Sin-paced starvation gaps so the ramp covers the attention tail.
  * No warm_exp: ACT is in-order, so the Exp table load begins right after
    the last Sin retires (during the exp's PSUM-sem wait) — a warm
    trigger only adds its own ~0.3us.
  * Final writeback qb1 split across the two idle trigger queues.
  * Residual fixed costs: ~7.7us engine-start preamble, ~2.2us kh DMA
    latency, ~2.5us walrus teardown epilogue.
"""
import math
import os
import numpy as np
import ml_dtypes

import concourse.bass as bass
import concourse.mybir as mybir
from concourse.tile import TileContext
from concourse.bass_utils import run_bass_kernel_spmd

F32 = mybir.dt.float32
BF16 = mybir.dt.bfloat16
FP16 = mybir.dt.float16
FP8 = mybir.dt.float8e4
I32 = mybir.dt.int32
AFT = mybir.ActivationFunctionType
ALU = mybir.AluOpType
PM = mybir.MatmulPerfMode
BF16NP = ml_dtypes.bfloat16
FP8NP = ml_dtypes.float8_e4m3

B, Q, K, D, H = 4, 256, 1024, 512, 128
N_CORES = 8
M_HARM = int(os.environ.get("KM", "8"))
NF = 2 * M_HARM
NF8 = NF - 2                                     # fp8 features (m >= 2)
PSCALE = float(os.environ.get("KPS", "1.0"))
WFLOOR = float(os.environ.get("KFL", "0.0003"))
QFSCALE = float(os.environ.get("KQFS", "64.0"))
SIN_GRAN = int(os.environ.get("KSGRAN", "3"))    # fp8 features per Sin instr
PWA = int(os.environ.get("KPWA", "7"))           # warm MMs before seeds
PWB = int(os.environ.get("KPWB", "6"))           # warm MMs after seeds
# filler warm MMs after the m1 MMs and after each fp8 group's MMs: keep the
# PE continuously busy through Sin-paced starvation gaps (p-state ramp)
WFILL = [int(x) for x in os.environ.get("KWFILL", "2,2,2,2,1,3").split(",")]

FB = 14
MAGIC = 1.5 * 2.0**23
KEEP_MASK = 0x4B000000 | ((1 << FB) - 1)
ACT_SIN_SCALE = -2.0 * math.pi / (1 << FB)
ACT_SIN_BIAS = 2.0 * math.pi * (2.0**23) / (1 << FB) + math.pi
PAD_BIAS = -30000.0
DIRECT_LIM = 3.0

_GRAPH_CACHE = {}


# ---------------------------------------------------------------------------
def _fix_multiwait(nc):
    ctr = 0
    for f in nc.m.functions:
        for bb in f.blocks:
            ins_list = bb.instructions
            if not any(
                len(i.sync_info.on_wait)
                > (2 if isinstance(i, mybir.InstEventSemaphore) else 1)
                for i in ins_list
                if getattr(i, "sync_info", None) is not None
            ):
                continue
            new_list = []
            for inst in ins_list:
                si = getattr(inst, "sync_info", None)
                if si is not None:
                    waits = list(si.on_wait)
                    cap = 2 if isinstance(inst, mybir.InstEventSemaphore) else 1
                    if len(waits) > cap:
                        extra = waits[cap:]
                        for kk in range(0, len(extra), 2):
                            es = mybir.InstEventSemaphore(
                                name=f"waitfix_{ctr}", engine=inst.engine
                            )
                            ctr += 1
                            es.sync_info = mybir.SyncInfo(
                                on_wait=extra[kk : kk + 2], on_update=[]
                            )
                            new_list.append(es)
                        inst.sync_info = mybir.SyncInfo(
                            on_wait=waits[:cap], on_update=list(si.on_update)
                        )
                new_list.append(inst)
            bb.instructions = new_list
    return nc


def _register_const(nc, val, dtype=F32):
    if (dtype, val) in nc.const_aps.aps:
        return
    t = nc.alloc_sbuf_tensor(f"constap-{len(nc.const_aps.aps)}", [128, 1], dtype)
    nc.gpsimd.memset(t.ap(), val)
    nc.const_aps.aps[(dtype, val)] = t.ap()


def _fit_fourier(S, sigma):
    omega = math.pi / (PSCALE * S)
    s = np.linspace(-S, S, 20001)
    A = np.sin(np.outer(s, omega * np.arange(1, M_HARM + 1)))
    w = np.sqrt(np.exp(-0.5 * (s / sigma) ** 2) + WFLOOR)
    c, *_ = np.linalg.lstsq(A * w[:, None], np.tanh(s) * w, rcond=None)
    return omega, c.astype(np.float64)


def _choose_slices(vls):
    """Smallest slice width w (multiple of 32) with sum ceil(vl/w) <= 8."""
    for w in range(32, K + 1, 32):
        if sum((int(vl) + w - 1) // w for vl in vls) <= N_CORES:
            slices = []
            for b, vl in enumerate(vls):
                vl = int(vl)
                for k0 in range(0, vl, w):
                    slices.append((b, k0, min(w, vl - k0)))
            return w, slices
    raise AssertionError("unreachable: w=K always fits")


# ---------------------------------------------------------------------------
def _build_graph(Ks, omega, j0_direct):
    ntk = (Ks + 127) // 128
    ktw = [min(128, Ks - 128 * t) for t in range(ntk)]
    nc = bass.Bass()
    _register_const(nc, ACT_SIN_BIAS)
    _register_const(nc, math.pi / 2)
    _register_const(nc, 0.0)

    # kh DRAM rows padded to a 1024B multiple: 576B rows produce 128
    # un-bundleable descriptors whose per-descriptor overhead drags the
    # critical first transfer by up to ~2us run-to-run
    Kpad = ((Ks * 2 + 1023) // 1024) * 512
    kh_ext = nc.declare_dram_parameter("kh", [H, Kpad], FP16, isOutput=False)
    qf1_ext = nc.declare_dram_parameter("qf1", [H, 2 * Q], BF16, isOutput=False)
    qf8_ext = nc.declare_dram_parameter("qf8", [H, NF8 * Q], FP8, isOutput=False)
    v_ext = nc.declare_dram_parameter("v", [Ks, D], BF16, isOutput=False)
    maskr_ext = nc.declare_dram_parameter("maskrow", [1, Ks], BF16, isOutput=False)
    ident_ext = nc.declare_dram_parameter("ident", [128, 128], BF16, isOutput=False)
    outnum_ext = nc.declare_dram_parameter("outnum", [Q, D + 1], BF16, isOutput=True)

    with TileContext(nc) as tc:
        with tc.tile_pool(name="io", bufs=1) as io, \
             tc.tile_pool(name="work", bufs=1) as work:
            warm_src = work.tile([1, 320], BF16)
            nc.gpsimd.memset(warm_src[:], 1.0)       # also the seed ones-row

            khp = io.tile([H, Kpad], FP16)
            nc.sync.dma_start(khp[:], kh_ext[:])
            qf1 = io.tile([H, 2 * Q], BF16)
            nc.sync.dma_start(qf1[:], qf1_ext[:])
            qf8 = io.tile([H, NF8, Q], FP8)
            half = NF8 // 2
            nc.sync.dma_start(
                qf8[:, :half, :],
                qf8_ext[:, : half * Q].rearrange("h (f q) -> h f q", f=half))
            nc.sync.dma_start(
                qf8[:, half:, :],
                qf8_ext[:, half * Q :].rearrange("h (f q) -> h f q", f=half))
            maskrow = io.tile([1, Ks], BF16)
            nc.gpsimd.dma_start(maskrow[:], maskr_ext[:])
            ident = io.tile([128, 128], BF16)
            nc.gpsimd.dma_start(ident[:], ident_ext[:])
            v_all = io.tile([128, ntk * D], BF16)
            if ntk > 1:
                nc.gpsimd.dma_start(
                    v_all[:, : (ntk - 1) * D].rearrange("p (a d) -> p a d", a=ntk - 1),
                    v_ext[: (ntk - 1) * 128, :].rearrange("(a p) d -> p a d", p=128))
            nc.gpsimd.dma_start(v_all[: ktw[-1], (ntk - 1) * D:],
                                v_ext[(ntk - 1) * 128:, :])

            kf1 = work.tile([H, 2 * Ks], BF16)       # m=1 K-side features
            kf8 = work.tile([H, NF8, Ks], FP8)       # m>=2 K-side features
            num_qk = work.tile([128, 2 * Ks], BF16)
            numT = work.tile([128, ntk * Q], BF16)
            den = work.tile([128, 2], F32)
            out_sb = work.tile([128, 2 * (D + 1)], BF16)

            with tc.tile_pool(name="pssc", bufs=1, space="PSUM") as pssc, \
                 tc.tile_pool(name="fwork", bufs=1) as fwp:
                ps = [pssc.tile([128, 512], F32, name=f"sc{qb}") for qb in range(2)]
                warm_ps = pssc.tile([128, 512], F32, name="warm_ps")

                def warm(n):
                    for _ in range(n):
                        nc.tensor.matmul(warm_ps[:, :Ks], warm_src[0:1, 0:128],
                                         warm_src[0:1, :Ks], start=True,
                                         stop=True, skip_group_check=True)

                warm(PWA)
                for qb in range(2):
                    nc.tensor.matmul(ps[qb][:, :Ks], warm_src[0:1, 0:128],
                                     maskrow[:], start=True, stop=False)
                warm(PWB)

                # ---- m=1 features (bf16).
                if j0_direct:
                    nc.scalar.activation(kf1[:, :Ks], khp[:, :Ks], AFT.Sin,
                                         scale=omega, bias=0.0)
                else:
                    wb1a = fwp.tile([H, Ks], F32, tag="wb1a")
                    c_fix = omega / (2 * math.pi) * (1 << FB)
                    nc.vector.tensor_scalar(wb1a[:], khp[:, :Ks], c_fix, MAGIC,
                                            ALU.mult, ALU.add)
                    nc.vector.tensor_scalar(wb1a[:].bitcast(I32),
                                            wb1a[:].bitcast(I32),
                                            KEEP_MASK, None, ALU.bitwise_and)
                    nc.scalar.activation(kf1[:, :Ks], wb1a[:], AFT.Sin,
                                         scale=ACT_SIN_SCALE, bias=ACT_SIN_BIAS)
                wb1 = fwp.tile([H, Ks], F32, tag="wb1")
                c_fix = omega / (2 * math.pi) * (1 << FB)
                nc.vector.tensor_scalar(wb1[:], khp[:, :Ks], c_fix,
                                        MAGIC + (1 << FB) / 4.0,
                                        ALU.mult, ALU.add)
                nc.vector.tensor_scalar(wb1[:].bitcast(I32), wb1[:].bitcast(I32),
                                        KEEP_MASK, None, ALU.bitwise_and)
                nc.scalar.activation(kf1[:, Ks:], wb1[:], AFT.Sin,
                                     scale=ACT_SIN_SCALE, bias=ACT_SIN_BIAS)
                for j in range(2):
                    for qb in range(2):
                        nc.tensor.matmul(
                            ps[qb][:, :Ks],
                            qf1[:, j * Q + qb * 128 : j * Q + (qb + 1) * 128],
                            kf1[:, j * Ks : (j + 1) * Ks],
                            start=False, stop=False)
                warm(WFILL[0])

                # ---- fp8 features in Sin groups; DoubleRow MM per pair.
                groups = []
                f = 0
                while f < NF8:
                    g = min(SIN_GRAN, NF8 - f)
                    groups.append((f, g))
                    f += g
                for gi, (f, g) in enumerate(groups):
                    wband = fwp.tile([H, SIN_GRAN, Ks], F32, tag="wband", bufs=3)
                    for i in range(g):
                        j = f + i + 2                  # global feature index
                        m = j // 2 + 1
                        is_cos = j % 2 == 1
                        c_fix = m * omega / (2 * math.pi) * (1 << FB)
                        add_c = MAGIC + ((1 << FB) / 4.0 if is_cos else 0.0)
                        nc.vector.tensor_scalar(wband[:, i, :], khp[:, :Ks],
                                                c_fix, add_c,
                                                ALU.mult, ALU.add)
                    seg = wband[:, :g, :].rearrange("p a b -> p (a b)")
                    nc.vector.tensor_scalar(seg.bitcast(I32), seg.bitcast(I32),
                                            KEEP_MASK, None, ALU.bitwise_and)
                    nc.scalar.activation(kf8[:, f : f + g, :], wband[:, :g, :],
                                         AFT.Sin, scale=ACT_SIN_SCALE,
                                         bias=ACT_SIN_BIAS)
                    for p in range(f // 2, (f + g) // 2):
                        for qb in range(2):
                            nc.tensor.matmul(
                                ps[qb][:, :Ks],
                                qf8[:, 2 * p : 2 * p + 2,
                                    qb * 128 : (qb + 1) * 128],
                                kf8[:, 2 * p : 2 * p + 2, :],
                                start=False, stop=(p == NF8 // 2 - 1),
                                perf_mode=PM.DoubleRow)
                    warm(WFILL[min(gi + 1, len(WFILL) - 1)])

                # ---- softmax numerators + denominators. ACT is in-order:
                # the Exp act-table load starts right after the last Sin
                # retires, overlapping the score-matmul drain wait.
                nc.scalar.activation(num_qk[:, :Ks], ps[0][:, :Ks], AFT.Exp,
                                     scale=1.0 / QFSCALE)
                nc.vector.tensor_reduce(den[:, 0:1], num_qk[:, :Ks],
                                        mybir.AxisListType.X, ALU.add)
                nc.scalar.activation(num_qk[:, Ks:], ps[1][:, :Ks], AFT.Exp,
                                     scale=1.0 / QFSCALE, accum_out=den[:, 1:2])

            # ---- transpose numerators, attention matmul, write-out.
            # tail k-tile order: the short (32-row) tile FIRST so the LAST
            # accumulating attention MM is a full-rate 128-row one (a short
            # MM after an idle gap runs at the dropped clock)
            t_ord = [ntk - 1] + list(range(ntk - 1))
            with tc.tile_pool(name="pstr", bufs=3, space="PSUM") as pstr, \
                 tc.tile_pool(name="psout", bufs=1, space="PSUM") as psout:
                out_ps = [psout.tile([128, D], F32, name=f"o{qb}") for qb in range(2)]
                for qb in range(2):
                    ob = qb * (D + 1)
                    nc.vector.tensor_copy(out_sb[:, ob + D : ob + D + 1],
                                          den[:, qb : qb + 1])
                for qb in range(2):
                    for i, t in enumerate(t_ord):
                        tr = pstr.tile([128, 128], BF16, tag="tr")
                        nc.tensor.transpose(
                            tr[: ktw[t], :],
                            num_qk[:, qb * Ks + t * 128 : qb * Ks + t * 128 + ktw[t]],
                            ident[:])
                        if i % 2 == 1:
                            nc.scalar.activation(
                                numT[: ktw[t], t * Q + qb * 128 : t * Q + (qb + 1) * 128],
                                tr[: ktw[t], :], AFT.Copy)
                        else:
                            nc.vector.tensor_copy(
                                numT[: ktw[t], t * Q + qb * 128 : t * Q + (qb + 1) * 128],
                                tr[: ktw[t], :])
                for qb in range(2):
                    for i, t in enumerate(t_ord):
                        nc.tensor.matmul(
                            out_ps[qb][:],
                            numT[: ktw[t], t * Q + qb * 128 : t * Q + (qb + 1) * 128],
                            v_all[: ktw[t], t * D : (t + 1) * D],
                            start=(i == 0), stop=(i == ntk - 1))
                    ob = qb * (D + 1)
                    # cast split across VE and ACT in parallel (~520ns vs
                    # VE-alone 694ns on the critical last chain)
                    nc.vector.tensor_copy(out_sb[:, ob : ob + 256],
                                          out_ps[qb][:, 0:256])
                    nc.scalar.activation(out_sb[:, ob + 256 : ob + D],
                                         out_ps[qb][:, 256:D], AFT.Copy)
                    if qb == 0:
                        nc.sync.dma_start(outnum_ext[0:128, :],
                                          out_sb[:, ob : ob + D + 1])
                    else:
                        # final writeback split across two idle queues
                        nc.scalar.dma_start(outnum_ext[128:192, :],
                                            out_sb[0:64, ob : ob + D + 1])
                        nc.gpsimd.dma_start(outnum_ext[192:256, :],
                                            out_sb[64:128, ob : ob + D + 1])
    return _fix_multiwait(nc)


# ---------------------------------------------------------------------------
def kernel(q, k, v, valid_lens, w_q, w_k, w_v):
    q = np.asarray(q, np.float32)
    k = np.asarray(k, np.float32)
    v = np.asarray(v, np.float32)
    w_q = np.asarray(w_q, np.float32)
    w_k = np.asarray(w_k, np.float32)
    w_v = np.asarray(w_v, np.float32)
    vls = np.asarray(valid_lens).astype(np.int64)

    qh = np.einsum("bqd,hd->bqh", q, w_q)          # [B,Q,H]
    kh = np.einsum("bkd,hd->bkh", k, w_k)          # [B,K,H]
    S = float(np.abs(qh).max() + np.abs(kh).max()) * 1.02 + 1e-3
    sigma = float(np.sqrt(qh.var() + kh.var()))
    omega, cm = _fit_fourier(S, sigma)
    c_shift = float(np.abs(w_v).sum()) + 0.5
    s_side = float(np.abs(kh).max()) * 1.02
    j0_direct = omega * s_side < DIRECT_LIM

    Ks, slices = _choose_slices(vls)

    key = (Ks, round(omega, 9), j0_direct)
    if key not in _GRAPH_CACHE:
        _GRAPH_CACHE[key] = _build_graph(Ks, omega, j0_direct)
    nc = _GRAPH_CACHE[key]

    # Q-side banks (scaled by QFSCALE; exp un-scales): K-feature j even=sin
    # pairs with Q cos, odd=cos pairs with Q sin.
    qf1_banks, qf8_banks = [], []
    for b in range(B):
        buf1 = np.empty((H, 2 * Q), dtype=BF16NP)
        buf8 = np.empty((H, NF8 * Q), dtype=FP8NP)
        for m in range(1, M_HARM + 1):
            a = omega * m * qh[b]                  # [Q,H]
            scale = (QFSCALE * w_v * cm[m - 1]).astype(np.float32)
            qc = (np.cos(a) * scale[None, :]).T
            qs = (np.sin(a) * scale[None, :]).T
            if m == 1:
                buf1[:, :Q] = qc.astype(BF16NP)
                buf1[:, Q:] = qs.astype(BF16NP)
            else:
                f = 2 * (m - 2)
                buf8[:, f * Q : (f + 1) * Q] = qc.astype(FP8NP)
                buf8[:, (f + 1) * Q : (f + 2) * Q] = qs.astype(FP8NP)
        qf1_banks.append(buf1)
        qf8_banks.append(buf8)

    ident = np.eye(128, dtype=BF16NP)
    in_maps = []
    core_slices = [slices[c % len(slices)] for c in range(N_CORES)]
    Kpad = ((Ks * 2 + 1023) // 1024) * 512
    for c in range(N_CORES):
        b, k0, kw = core_slices[c]
        khs = np.zeros((H, Kpad), np.float16)
        khs[:, :kw] = kh[b, k0 : k0 + kw, :].T.astype(np.float16)
        vs = np.zeros((Ks, D), BF16NP)
        vs[:kw] = v[b, k0 : k0 + kw, :].astype(BF16NP)
        maskrow = np.full((1, Ks), QFSCALE * PAD_BIAS, np.float32)
        maskrow[0, :kw] = -QFSCALE * c_shift
        in_maps.append({
            "kh": khs,
            "qf1": qf1_banks[b],
            "qf8": qf8_banks[b],
            "v": vs,
            "maskrow": maskrow.astype(BF16NP),
            "ident": ident,
        })

    res = run_bass_kernel_spmd(nc, in_maps, core_ids=list(range(N_CORES)))

    acc = np.zeros((B, Q, D + 1), np.float64)
    for c in range(len(slices)):
        acc[core_slices[c][0]] += res.results[c]["outnum"]
    return (acc[:, :, :D] / acc[:, :, D:]).astype(np.float32)


# revision 22
# speedup vs baseline: 1.0581x; 1.0581x over previous
"""Additive (Bahdanau) attention on 8 TRN2 NeuronCores.

Reference (B=4, Q=256, K=1024, D=512, H=128):
    qh = q @ w_q.T; kh = k @ w_k.T
    scores[b,q,k] = sum_h w_v[h] * tanh(qh[b,q,h] + kh[b,k,h])
    scores masked to -1e6 for k >= valid_lens[b]; softmax over k; out = attn @ v.

Fourier factorization: tanh(s) ~= sum_{m<=M} c_m sin(m*w*s) (weighted lstsq
fit on the data range); the angle-sum identity makes the [Q,K,H] elementwise
tensor separable into per-harmonic sin/cos features contracted over H on the
PE. M=8 with (PSCALE=1.0, WFLOOR=3e-4): rel err 1.127e-2 measured ==
emulated (gate 2e-2; M=9 lands 9.2e-3 at ~+0.7us).

Key structure (v1 32.6us -> 30.3us / 29.3us mean):
  * K-sliced sharding: valid key ranges cut into <=8 width-288 slices; each
    core computes full-Q bf16 partials [256, 513] = [num | den]; host sums
    slices per batch in fp64 and divides. Softmax shift is the constant
    c_shift so partial exp sums combine exactly.
  * fp8 DoubleRow matmuls for harmonics m>=2: Q/K feature (sin,cos) pairs
    packed along the middle dim of [H, 2, *] APs; one MM per (pair, qblock)
    does a 256-deep contraction in 288 column-cycles (2x bf16). m=1 stays
    bf16 (largest c_m dominates the noise budget). qf pre-scaled by 64 to
    clear the e4m3 denormal floor; exp un-scales via its scale immediate.
  * K-side features built on device: VE fixed-point range reduction
    (u = x*c+MAGIC rounds in the fp32 mantissa; AND keeps exponent + low
    FB bits; ACT Sin's scale/bias maps back to [-pi,pi]). Host-side fp16
    phase shipping was tried and is SLOWER: 1.3MB of phases through the 3
    trigger queues (~190GB/s each) DMA-starves the Sin chain.
  * Activation bias consts are raw (untracked) SBUF tensors memset on
    gpsimd: a Tile-tracked bias adds a second wait-sem per Sin, which the
    multiwait fix turns into queue-stalling EventSemaphores (blocks the
    head Sin's act-table load from overlapping the kh DMA wait). The
    memsets retire >1.4us before the first possible ACT bias read.
  * DMA triggers on sync/gpsimd only (scalar carries none early): ACT's
    first queue entry is the first Sin, so the Sin act-table load runs at
    ~6.7us, inside the kh DMA latency.
  * PE p-state: the clock ramps to 2.4GHz only after a long CONTINUOUS
    busy run (~11us from first PE op); idle gaps reset it. Warm matmuls
    on a memset tile start the PE at ~7.2us; filler warms plug the
    Sin-paced starvation gaps so the ramp covers the attention tail.
  * No warm_exp: ACT is in-order, so the Exp table load begins right after
    the last Sin retires (during the exp's PSUM-sem wait) — a warm
    trigger only adds its own ~0.3us.
  * Final writeback qb1 split across the two idle trigger queues.
  * Residual fixed costs: ~7.7us engine-start preamble, ~2.2us kh DMA
    latency, ~2.5us walrus teardown epilogue.
"""
import math
import os
import numpy as np
import ml_dtypes

import concourse.bass as bass
import concourse.mybir as mybir
from concourse.tile import TileContext
from concourse.bass_utils import run_bass_kernel_spmd

F32 = mybir.dt.float32
BF16 = mybir.dt.bfloat16
FP16 = mybir.dt.float16
FP8 = mybir.dt.float8e4
I32 = mybir.dt.int32
AFT = mybir.ActivationFunctionType
ALU = mybir.AluOpType
PM = mybir.MatmulPerfMode
BF16NP = ml_dtypes.bfloat16
FP8NP = ml_dtypes.float8_e4m3

B, Q, K, D, H = 4, 256, 1024, 512, 128
N_CORES = 8
M_HARM = int(os.environ.get("KM", "8"))
NF = 2 * M_HARM
NF8 = NF - 2                                     # fp8 features (m >= 2)
PSCALE = float(os.environ.get("KPS", "1.0"))
WFLOOR = float(os.environ.get("KFL", "0.0003"))
QFSCALE = float(os.environ.get("KQFS", "64.0"))
SIN_GRAN = int(os.environ.get("KSGRAN", "3"))    # fp8 features per Sin instr
PWA = int(os.environ.get("KPWA", "7"))           # warm MMs before seeds
PWB = int(os.environ.get("KPWB", "6"))           # warm MMs after seeds
# filler warm MMs after the m1 MMs and after each fp8 group's MMs: keep the
# PE continuously busy through Sin-paced starvation gaps (p-state ramp)
WFILL = [int(x) for x in os.environ.get("KWFILL", "2,2,2,2,1,0").split(",")]

FB = 14
MAGIC = 1.5 * 2.0**23
KEEP_MASK = 0x4B000000 | ((1 << FB) - 1)
ACT_SIN_SCALE = -2.0 * math.pi / (1 << FB)
ACT_SIN_BIAS = 2.0 * math.pi * (2.0**23) / (1 << FB) + math.pi
PAD_BIAS = -30000.0
DIRECT_LIM = 3.0

_GRAPH_CACHE = {}


# ---------------------------------------------------------------------------
def _fix_multiwait(nc):
    ctr = 0
    for f in nc.m.functions:
        for bb in f.blocks:
            ins_list = bb.instructions
            if not any(
                len(i.sync_info.on_wait)
                > (2 if isinstance(i, mybir.InstEventSemaphore) else 1)
                for i in ins_list
                if getattr(i, "sync_info", None) is not None
            ):
                continue
            new_list = []
            for inst in ins_list:
                si = getattr(inst, "sync_info", None)
                if si is not None:
                    waits = list(si.on_wait)
                    cap = 2 if isinstance(inst, mybir.InstEventSemaphore) else 1
                    if len(waits) > cap:
                        extra = waits[cap:]
                        for kk in range(0, len(extra), 2):
                            es = mybir.InstEventSemaphore(
                                name=f"waitfix_{ctr}", engine=inst.engine
                            )
                            ctr += 1
                            es.sync_info = mybir.SyncInfo(
                                on_wait=extra[kk : kk + 2], on_update=[]
                            )
                            new_list.append(es)
                        inst.sync_info = mybir.SyncInfo(
                            on_wait=waits[:cap], on_update=list(si.on_update)
                        )
                new_list.append(inst)
            bb.instructions = new_list
    return nc


def _register_const(nc, val, dtype=F32):
    if (dtype, val) in nc.const_aps.aps:
        return
    t = nc.alloc_sbuf_tensor(f"constap-{len(nc.const_aps.aps)}", [128, 1], dtype)
    nc.gpsimd.memset(t.ap(), val)
    nc.const_aps.aps[(dtype, val)] = t.ap()


def _fit_fourier(S, sigma):
    omega = math.pi / (PSCALE * S)
    s = np.linspace(-S, S, 20001)
    A = np.sin(np.outer(s, omega * np.arange(1, M_HARM + 1)))
    w = np.sqrt(np.exp(-0.5 * (s / sigma) ** 2) + WFLOOR)
    c, *_ = np.linalg.lstsq(A * w[:, None], np.tanh(s) * w, rcond=None)
    return omega, c.astype(np.float64)


def _choose_slices(vls):
    """Smallest slice width w (multiple of 32) with sum ceil(vl/w) <= 8."""
    for w in range(32, K + 1, 32):
        if sum((int(vl) + w - 1) // w for vl in vls) <= N_CORES:
            slices = []
            for b, vl in enumerate(vls):
                vl = int(vl)
                for k0 in range(0, vl, w):
                    slices.append((b, k0, min(w, vl - k0)))
            return w, slices
    raise AssertionError("unreachable: w=K always fits")


# ---------------------------------------------------------------------------
def _build_graph(Ks, omega, j0_direct):
    ntk = (Ks + 127) // 128
    ktw = [min(128, Ks - 128 * t) for t in range(ntk)]
    nc = bass.Bass()
    _register_const(nc, ACT_SIN_BIAS)
    _register_const(nc, math.pi / 2)
    _register_const(nc, 0.0)

    # kh DRAM rows padded to a 1024B multiple: 576B rows produce 128
    # un-bundleable descriptors whose per-descriptor overhead drags the
    # critical first transfer by up to ~2us run-to-run
    Kpad = ((Ks * 2 + 1023) // 1024) * 512
    kh_ext = nc.declare_dram_parameter("kh", [H, Kpad], FP16, isOutput=False)
    qf1_ext = nc.declare_dram_parameter("qf1", [H, 2 * Q], BF16, isOutput=False)
    qf8_ext = nc.declare_dram_parameter("qf8", [H, NF8 * Q], FP8, isOutput=False)
    v_ext = nc.declare_dram_parameter("v", [Ks, D], BF16, isOutput=False)
    maskr_ext = nc.declare_dram_parameter("maskrow", [1, Ks], BF16, isOutput=False)
    ident_ext = nc.declare_dram_parameter("ident", [128, 128], BF16, isOutput=False)
    outnum_ext = nc.declare_dram_parameter("outnum", [Q, D + 1], BF16, isOutput=True)

    with TileContext(nc) as tc:
        with tc.tile_pool(name="io", bufs=1) as io, \
             tc.tile_pool(name="work", bufs=1) as work:
            warm_src = work.tile([1, 320], BF16)
            nc.gpsimd.memset(warm_src[:], 1.0)       # also the seed ones-row

            khp = io.tile([H, Kpad], FP16)
            nc.sync.dma_start(khp[:], kh_ext[:])
            qf1 = io.tile([H, 2 * Q], BF16)
            nc.sync.dma_start(qf1[:], qf1_ext[:])
            qf8 = io.tile([H, NF8, Q], FP8)
            half = NF8 // 2
            nc.sync.dma_start(
                qf8[:, :half, :],
                qf8_ext[:, : half * Q].rearrange("h (f q) -> h f q", f=half))
            nc.sync.dma_start(
                qf8[:, half:, :],
                qf8_ext[:, half * Q :].rearrange("h (f q) -> h f q", f=half))
            maskrow = io.tile([1, Ks], BF16)
            nc.gpsimd.dma_start(maskrow[:], maskr_ext[:])
            ident = io.tile([128, 128], BF16)
            nc.gpsimd.dma_start(ident[:], ident_ext[:])
            v_all = io.tile([128, ntk * D], BF16)
            if ntk > 1:
                nc.gpsimd.dma_start(
                    v_all[:, : (ntk - 1) * D].rearrange("p (a d) -> p a d", a=ntk - 1),
                    v_ext[: (ntk - 1) * 128, :].rearrange("(a p) d -> p a d", p=128))
            nc.gpsimd.dma_start(v_all[: ktw[-1], (ntk - 1) * D:],
                                v_ext[(ntk - 1) * 128:, :])

            kf1 = work.tile([H, 2 * Ks], BF16)       # m=1 K-side features
            kf8 = work.tile([H, NF8, Ks], FP8)       # m>=2 K-side features
            num_qk = work.tile([128, 2 * Ks], BF16)
            numT = work.tile([128, ntk * Q], BF16)
            den = work.tile([128, 2], F32)
            out_sb = work.tile([128, 2 * (D + 1)], BF16)

            # all PSUM pools open CONCURRENTLY (7 tiles <= 8 banks): if the
            # tail pools open after pssc closes, the allocator reuses the
            # score-PSUM banks and the resulting anti-dependency semaphores
            # serialize the transposes/attention MMs behind the exps' PSUM
            # reads (~0.9us of tail idle in the v6 trace)
            with tc.tile_pool(name="pssc", bufs=1, space="PSUM") as pssc, \
                 tc.tile_pool(name="fwork", bufs=1) as fwp, \
                 tc.tile_pool(name="pstr", bufs=2, space="PSUM") as pstr, \
                 tc.tile_pool(name="psout", bufs=1, space="PSUM") as psout:
                ps = [pssc.tile([128, 512], F32, name=f"sc{qb}") for qb in range(2)]
                warm_ps = pssc.tile([128, 512], F32, name="warm_ps")

                def warm(n):
                    for _ in range(n):
                        nc.tensor.matmul(warm_ps[:, :Ks], warm_src[0:1, 0:128],
                                         warm_src[0:1, :Ks], start=True,
                                         stop=True, skip_group_check=True)

                warm(PWA)
                for qb in range(2):
                    nc.tensor.matmul(ps[qb][:, :Ks], warm_src[0:1, 0:128],
                                     maskrow[:], start=True, stop=False)
                warm(PWB)

                # ---- m=1 features (bf16).
                if j0_direct:
                    nc.scalar.activation(kf1[:, :Ks], khp[:, :Ks], AFT.Sin,
                                         scale=omega, bias=0.0)
                else:
                    wb1a = fwp.tile([H, Ks], F32, tag="wb1a")
                    c_fix = omega / (2 * math.pi) * (1 << FB)
                    nc.vector.tensor_scalar(wb1a[:], khp[:, :Ks], c_fix, MAGIC,
                                            ALU.mult, ALU.add)
                    nc.vector.tensor_scalar(wb1a[:].bitcast(I32),
                                            wb1a[:].bitcast(I32),
                                            KEEP_MASK, None, ALU.bitwise_and)
                    nc.scalar.activation(kf1[:, :Ks], wb1a[:], AFT.Sin,
                                         scale=ACT_SIN_SCALE, bias=ACT_SIN_BIAS)
                wb1 = fwp.tile([H, Ks], F32, tag="wb1")
                c_fix = omega / (2 * math.pi) * (1 << FB)
                nc.vector.tensor_scalar(wb1[:], khp[:, :Ks], c_fix,
                                        MAGIC + (1 << FB) / 4.0,
                                        ALU.mult, ALU.add)
                nc.vector.tensor_scalar(wb1[:].bitcast(I32), wb1[:].bitcast(I32),
                                        KEEP_MASK, None, ALU.bitwise_and)
                nc.scalar.activation(kf1[:, Ks:], wb1[:], AFT.Sin,
                                     scale=ACT_SIN_SCALE, bias=ACT_SIN_BIAS)
                for j in range(2):
                    for qb in range(2):
                        nc.tensor.matmul(
                            ps[qb][:, :Ks],
                            qf1[:, j * Q + qb * 128 : j * Q + (qb + 1) * 128],
                            kf1[:, j * Ks : (j + 1) * Ks],
                            start=False, stop=False)
                warm(WFILL[0])

                # ---- fp8 features in Sin groups; DoubleRow MM per pair.
                groups = []
                f = 0
                while f < NF8:
                    g = min(SIN_GRAN, NF8 - f)
                    groups.append((f, g))
                    f += g
                for gi, (f, g) in enumerate(groups):
                    wband = fwp.tile([H, SIN_GRAN, Ks], F32, tag="wband", bufs=3)
                    for i in range(g):
                        j = f + i + 2                  # global feature index
                        m = j // 2 + 1
                        is_cos = j % 2 == 1
                        c_fix = m * omega / (2 * math.pi) * (1 << FB)
                        add_c = MAGIC + ((1 << FB) / 4.0 if is_cos else 0.0)
                        nc.vector.tensor_scalar(wband[:, i, :], khp[:, :Ks],
                                                c_fix, add_c,
                                                ALU.mult, ALU.add)
                    seg = wband[:, :g, :].rearrange("p a b -> p (a b)")
                    nc.vector.tensor_scalar(seg.bitcast(I32), seg.bitcast(I32),
                                            KEEP_MASK, None, ALU.bitwise_and)
                    nc.scalar.activation(kf8[:, f : f + g, :], wband[:, :g, :],
                                         AFT.Sin, scale=ACT_SIN_SCALE,
                                         bias=ACT_SIN_BIAS)
                    for p in range(f // 2, (f + g) // 2):
                        for qb in range(2):
                            nc.tensor.matmul(
                                ps[qb][:, :Ks],
                                qf8[:, 2 * p : 2 * p + 2,
                                    qb * 128 : (qb + 1) * 128],
                                kf8[:, 2 * p : 2 * p + 2, :],
                                start=False, stop=(p == NF8 // 2 - 1),
                                perf_mode=PM.DoubleRow)
                    warm(WFILL[min(gi + 1, len(WFILL) - 1)])

                # ---- softmax numerators + denominators. ACT is in-order:
                # the Exp act-table load starts right after the last Sin
                # retires, overlapping the score-matmul drain wait.
                nc.scalar.activation(num_qk[:, :Ks], ps[0][:, :Ks], AFT.Exp,
                                     scale=1.0 / QFSCALE)
                nc.vector.tensor_reduce(den[:, 0:1], num_qk[:, :Ks],
                                        mybir.AxisListType.X, ALU.add)
                nc.scalar.activation(num_qk[:, Ks:], ps[1][:, :Ks], AFT.Exp,
                                     scale=1.0 / QFSCALE, accum_out=den[:, 1:2])

                # ---- transpose numerators, attention matmul, write-out.
                out_ps = [psout.tile([128, D], F32, name=f"o{qb}") for qb in range(2)]
                for qb in range(2):
                    ob = qb * (D + 1)
                    nc.vector.tensor_copy(out_sb[:, ob + D : ob + D + 1],
                                          den[:, qb : qb + 1])
                for qb in range(2):
                    for t in range(ntk):
                        tr = pstr.tile([128, 128], BF16, tag="tr")
                        nc.tensor.transpose(
                            tr[: ktw[t], :],
                            num_qk[:, qb * Ks + t * 128 : qb * Ks + t * 128 + ktw[t]],
                            ident[:])
                        if t % 2 == 1:
                            nc.scalar.activation(
                                numT[: ktw[t], t * Q + qb * 128 : t * Q + (qb + 1) * 128],
                                tr[: ktw[t], :], AFT.Copy)
                        else:
                            nc.vector.tensor_copy(
                                numT[: ktw[t], t * Q + qb * 128 : t * Q + (qb + 1) * 128],
                                tr[: ktw[t], :])
                for qb in range(2):
                    for t in range(ntk):
                        nc.tensor.matmul(
                            out_ps[qb][:],
                            numT[: ktw[t], t * Q + qb * 128 : t * Q + (qb + 1) * 128],
                            v_all[: ktw[t], t * D : (t + 1) * D],
                            start=(t == 0), stop=(t == ntk - 1))
                    ob = qb * (D + 1)
                    nc.vector.tensor_copy(out_sb[:, ob : ob + D], out_ps[qb][:])
                    if qb == 0:
                        nc.sync.dma_start(outnum_ext[0:128, :],
                                          out_sb[:, ob : ob + D + 1])
                    else:
                        # final writeback split across two idle queues
                        nc.scalar.dma_start(outnum_ext[128:192, :],
                                            out_sb[0:64, ob : ob + D + 1])
                        nc.gpsimd.dma_start(outnum_ext[192:256, :],
                                            out_sb[64:128, ob : ob + D + 1])
    return _fix_multiwait(nc)


# ---------------------------------------------------------------------------
def kernel(q, k, v, valid_lens, w_q, w_k, w_v):
    q = np.asarray(q, np.float32)
    k = np.asarray(k, np.float32)
    v = np.asarray(v, np.float32)
    w_q = np.asarray(w_q, np.float32)
    w_k = np.asarray(w_k, np.float32)
    w_v = np.asarray(w_v, np.float32)
    vls = np.asarray(valid_lens).astype(np.int64)

    qh = np.einsum("bqd,hd->bqh", q, w_q)          # [B,Q,H]
    kh = np.einsum("bkd,hd->bkh", k, w_k)          # [B,K,H]
    S = float(np.abs(qh).max() + np.abs(kh).max()) * 1.02 + 1e-3
    sigma = float(np.sqrt(qh.var() + kh.var()))
    omega, cm = _fit_fourier(S, sigma)
    c_shift = float(np.abs(w_v).sum()) + 0.5
    s_side = float(np.abs(kh).max()) * 1.02
    j0_direct = omega * s_side < DIRECT_LIM

    Ks, slices = _choose_slices(vls)

    key = (Ks, round(omega, 9), j0_direct)
    if key not in _GRAPH_CACHE:
        _GRAPH_CACHE[key] = _build_graph(Ks, omega, j0_direct)
    nc = _GRAPH_CACHE[key]

    # Q-side banks (scaled by QFSCALE; exp un-scales): K-feature j even=sin
    # pairs with Q cos, odd=cos pairs with Q sin.
    qf1_banks, qf8_banks = [], []
    for b in range(B):
        buf1 = np.empty((H, 2 * Q), dtype=BF16NP)
        buf8 = np.empty((H, NF8 * Q), dtype=FP8NP)
        for m in range(1, M_HARM + 1):
            a = omega * m * qh[b]                  # [Q,H]
            scale = (QFSCALE * w_v * cm[m - 1]).astype(np.float32)
            qc = (np.cos(a) * scale[None, :]).T
            qs = (np.sin(a) * scale[None, :]).T
            if m == 1:
                buf1[:, :Q] = qc.astype(BF16NP)
                buf1[:, Q:] = qs.astype(BF16NP)
            else:
                f = 2 * (m - 2)
                buf8[:, f * Q : (f + 1) * Q] = qc.astype(FP8NP)
                buf8[:, (f + 1) * Q : (f + 2) * Q] = qs.astype(FP8NP)
        qf1_banks.append(buf1)
        qf8_banks.append(buf8)

    ident = np.eye(128, dtype=BF16NP)
    in_maps = []
    core_slices = [slices[c % len(slices)] for c in range(N_CORES)]
    Kpad = ((Ks * 2 + 1023) // 1024) * 512
    for c in range(N_CORES):
        b, k0, kw = core_slices[c]
        khs = np.zeros((H, Kpad), np.float16)
        khs[:, :kw] = kh[b, k0 : k0 + kw, :].T.astype(np.float16)
        vs = np.zeros((Ks, D), BF16NP)
        vs[:kw] = v[b, k0 : k0 + kw, :].astype(BF16NP)
        maskrow = np.full((1, Ks), QFSCALE * PAD_BIAS, np.float32)
        maskrow[0, :kw] = -QFSCALE * c_shift
        in_maps.append({
            "kh": khs,
            "qf1": qf1_banks[b],
            "qf8": qf8_banks[b],
            "v": vs,
            "maskrow": maskrow.astype(BF16NP),
            "ident": ident,
        })

    res = run_bass_kernel_spmd(nc, in_maps, core_ids=list(range(N_CORES)))

    acc = np.zeros((B, Q, D + 1), np.float64)
    for c in range(len(slices)):
        acc[core_slices[c][0]] += res.results[c]["outnum"]
    return (acc[:, :, :D] / acc[:, :, D:]).astype(np.float32)
